# revision 13
# baseline (speedup 1.0000x reference)
"""Causal self-attention with ALiBi on 8 trn2 cores.

Sharding: data-parallel over batch (2) x tensor-parallel over head groups (4).
Core c handles batch b = c // 4, head group g = c % 4 (heads 4g..4g+3).
Each core computes qT/kT/v projections for its 4 heads, flash-style causal
attention with ALiBi folded into the score matmul via 2 augmented K rows
(k_aug = [iota_j; ones], q_aug = [slope; -slope*i]), and a partial output
projection.  Host sums the 4 partials per batch and adds bo.

v2 changes vs the 258us baseline:
- bf16 operands for every matmul except scores (x, wq/wk/wv/wo, v, exp(s), y).
  Scores stay f32r: the aug rows carry position values up to 2047 whose
  rounding error would multiply exp() directly.  End-to-end bf16 error
  measured 4e-3 vs the 2e-2 gate.  Halves x/w/out DMA bytes and PE power
  (the baseline trace showed 79us of power throttling at ~69% util limit).
- Attention for block qb is emitted right after the projections of block qb
  instead of after all four blocks, and independent matmuls (next block's
  projections, previous block's output projection) are woven between
  attention chunks.  The PE is in-order: every exp-wait stall both idles it
  and resets the 2.4GHz p-state ramp (1.2GHz for the next 3us), which is
  where the baseline lost ~60us.
- Softmax denominators: reciprocal_approx_fast in place at the denominator
  row (the baseline burned 27us of DVE in full-width `reciprocal` plus row
  shuffles).
"""

import sys

sys.path.insert(0, "/opt/trn_rl_repo")

import numpy as np

import concourse.bacc as bacc
import concourse.mybir as mybir
import concourse.tile as tile
from concourse.bass import ds, ts
from concourse.bass_utils import run_bass_kernel_spmd

B, T, D, H, DH = 2, 2048, 1024, 16, 64
G = 4            # head groups (tensor-parallel)
HPC = H // G     # heads per core
DG = D // G      # model dims per core (256)
P = 128
N_CORES = 8
NEG = -1.0e30

F32 = mybir.dt.float32
F32R = mybir.dt.float32r
BF16 = mybir.dt.bfloat16
ADD = mybir.AluOpType.add
MULT = mybir.AluOpType.mult
EXP = mybir.ActivationFunctionType.Exp

TRACE = False
DEBUG = False
LAST_RESULTS = None

_cache = {}


def _build(with_bias: bool):
    nc = bacc.Bacc("TRN2", target_bir_lowering=False, debug=False)

    xT_d = nc.dram_tensor("xT", [P, 8, T], BF16, kind="ExternalInput").ap()
    wq_d = nc.dram_tensor("wqT", [P, 8, DG], BF16, kind="ExternalInput").ap()
    wk_d = nc.dram_tensor("wkT", [P, 8, DG], BF16, kind="ExternalInput").ap()
    wv_d = nc.dram_tensor("wvT", [P, 8, DG], BF16, kind="ExternalInput").ap()
    wo_d = nc.dram_tensor("woT", [P, 2, D], BF16, kind="ExternalInput").ap()
    qaug_d = nc.dram_tensor("qaug", [HPC, 2, T], F32, kind="ExternalInput").ap()
    kaug_d = nc.dram_tensor("kaug", [2, T], F32, kind="ExternalInput").ap()
    ident_d = nc.dram_tensor("ident", [P, P], BF16, kind="ExternalInput").ap()
    maskst_d = nc.dram_tensor("maskst", [P, P], BF16, kind="ExternalInput").ap()
    if with_bias:
        bvo_d = nc.dram_tensor("bvo", [P, DG], F32, kind="ExternalInput").ap()
        bq_d = nc.dram_tensor("bq2", [P, 2], F32, kind="ExternalInput").ap()
        bk_d = nc.dram_tensor("bk2", [P, 2], F32, kind="ExternalInput").ap()
    out_d = nc.dram_tensor("outT", [P, 8, T], BF16, kind="ExternalOutput").ap()
    if DEBUG:
        qdump_d = nc.dram_tensor("qdump", [HPC, 66, T], F32, kind="ExternalOutput").ap()
        kdump_d = nc.dram_tensor("kdump", [HPC, 66, T], F32, kind="ExternalOutput").ap()
        vdump_d = nc.dram_tensor("vdump", [HPC, P, 16, P], BF16, kind="ExternalOutput").ap()
        ydump_d = nc.dram_tensor("ydump", [2, P, T], BF16, kind="ExternalOutput").ap()
        xdump_d = nc.dram_tensor("xdump", [4, P, 8, 512], BF16, kind="ExternalOutput").ap()
    dscr_d = nc.dram_tensor("dscratch", [16, 512], F32).ap()

    with tile.TileContext(nc) as tc:
        with (
            tc.tile_pool(name="big", bufs=1) as big,
            tc.tile_pool(name="stage", bufs=3) as stage,
            tc.tile_pool(name="obp", bufs=2) as obp,
            tc.tile_pool(name="expp", bufs=4) as expp,
            tc.tile_pool(name="small", bufs=3) as small,
            tc.tile_pool(name="pj", bufs=2, space="PSUM") as pjp,
            tc.tile_pool(name="pss", bufs=3, space="PSUM") as pssp,
            tc.tile_pool(name="psy", bufs=3, space="PSUM") as psyp,
        ):
            # ---- persistent tiles
            xb = [
                big.tile([P, 8, 512], BF16, tag=f"x{tq}", name=f"x{tq}")
                for tq in range(4)
            ]
            wv = big.tile([P, 8, DG], BF16, tag="wv", name="wv")
            wq = big.tile([P, 8, DG], BF16, tag="wq", name="wq")
            wk = big.tile([P, 8, DG], BF16, tag="wk", name="wk")
            wo = big.tile([P, 2, D], BF16, tag="wo", name="wo")
            qa = [big.tile([66, T], F32R, tag=f"qa{h}", name=f"qa{h}") for h in range(HPC)]
            ka = [big.tile([66, T], F32R, tag=f"ka{h}", name=f"ka{h}") for h in range(HPC)]
            va = [big.tile([P, 16, P], BF16, tag=f"va{h}", name=f"va{h}") for h in range(HPC)]
            yt = [big.tile([P, T], BF16, tag=f"yt{m}", name=f"yt{m}") for m in range(2)]

            # ---- loads. One packed DMA per x block / weight tensor (many
            # small dep-free DMAs overflow the queue ring slots and get
            # dropped).  Block 0 split into 4 column chunks so the first
            # v-projection starts as early as possible; first-needed weights
            # interleave with them.  Blocks 1-3 on gpsimd.
            nc.sync.dma_start(out=xb[0][:, :, 0:128], in_=xT_d[:, :, 0:128])
            nc.sync.dma_start(out=wv[:], in_=wv_d[:])
            nc.sync.dma_start(out=xb[0][:, :, 128:256], in_=xT_d[:, :, 128:256])
            nc.sync.dma_start(out=wq[:], in_=wq_d[:])
            nc.sync.dma_start(out=xb[0][:, :, 256:384], in_=xT_d[:, :, 256:384])
            nc.sync.dma_start(out=wk[:], in_=wk_d[:])
            nc.sync.dma_start(out=xb[0][:, :, 384:512], in_=xT_d[:, :, 384:512])
            ident_sb = big.tile([P, P], BF16, tag="ident")
            nc.sync.dma_start(out=ident_sb[:], in_=ident_d[:])
            maskst_sb = big.tile([P, P], BF16, tag="maskst")
            nc.sync.dma_start(out=maskst_sb[:], in_=maskst_d[:])
            if with_bias:
                bvo = big.tile([P, DG], F32, tag="bvo")
                nc.sync.dma_start(out=bvo[:], in_=bvo_d[:])
                bq2 = big.tile([P, 2], F32, tag="bq2")
                nc.sync.dma_start(out=bq2[:], in_=bq_d[:])
                bk2 = big.tile([P, 2], F32, tag="bk2")
                nc.sync.dma_start(out=bk2[:], in_=bk_d[:])
            for h in range(HPC):
                nc.sync.dma_start(out=qa[h][64:66, :], in_=qaug_d[h].bitcast(F32R))
                nc.sync.dma_start(out=ka[h][64:66, :], in_=kaug_d[:].bitcast(F32R))
                # ones column for the in-matmul softmax denominator; the odd
                # head's lands at partition 32 (engine APs need 32-aligned base)
                oc = 64 if h % 2 == 0 else 32
                for ch in range(16):
                    nc.vector.memset(va[h][:, ch, oc : oc + 1], 1.0)
            nc.sync.dma_start(out=wo[:], in_=wo_d[:])
            for tq in range(1, 4):
                nc.gpsimd.dma_start(
                    out=xb[tq][:], in_=xT_d[:, :, ts(tq, 512)]
                )

            # ---- emission units.  Each is (pe_cost_estimate, closure); the
            # weave below merges the attention backbone with independent
            # filler matmuls so the in-order PE stream never sits on an
            # exp-wait (stalls also drop the p-state to 1.2GHz).
            def unit_v(tq, ch):
                def emit():
                    lc = (ch % 4) * P
                    pv = pjp.tile([P, DG], F32, tag="pj", name=f"pv{4 * tq + ch}")
                    for kc in range(8):
                        nc.tensor.matmul(
                            out=pv[:],
                            lhsT=xb[tq][:, kc, lc : lc + P],
                            rhs=wv[:, kc, :],
                            start=(kc == 0),
                            stop=(kc == 7),
                        )
                    chg = 4 * tq + ch
                    for h in range(HPC):
                        off = 0 if h % 2 == 0 else 64
                        if with_bias:
                            nc.vector.tensor_tensor(
                                out=va[h][:, chg, off : off + 64],
                                in0=pv[:, h * 64 : h * 64 + 64],
                                in1=bvo[:, h * 64 : h * 64 + 64],
                                op=ADD,
                            )
                        else:
                            nc.vector.tensor_copy(
                                out=va[h][:, chg, off : off + 64],
                                in_=pv[:, h * 64 : h * 64 + 64],
                            )

                return (1100, emit)

            def unit_qk(tq, which, mc):
                def emit():
                    wt, dst = (wq, qa) if which == "q" else (wk, ka)
                    bt = None
                    if with_bias:
                        bt = bq2 if which == "q" else bk2
                    pq = pjp.tile([P, 512], F32, tag="pj", name=f"p{which}{tq}_{mc}")
                    for kc in range(8):
                        nc.tensor.matmul(
                            out=pq[:],
                            lhsT=wt[:, kc, ds(mc * P, P)],
                            rhs=xb[tq][:, kc, :],
                            start=(kc == 0),
                            stop=(kc == 7),
                        )
                    h_even, h_odd = 2 * mc, 2 * mc + 1
                    if with_bias:
                        nc.vector.tensor_scalar(
                            out=dst[h_even][0:64, ts(tq, 512)],
                            in0=pq[0:64, :],
                            scalar1=bt[0:64, mc : mc + 1],
                            scalar2=None,
                            op0=ADD,
                        )
                    else:
                        nc.vector.tensor_copy(
                            out=dst[h_even][0:64, ts(tq, 512)], in_=pq[0:64, :]
                        )
                    st = stage.tile([P, 512], F32R, tag="stage", name="st")
                    if with_bias:
                        nc.vector.tensor_scalar(
                            out=st[64:128, :],
                            in0=pq[64:128, :],
                            scalar1=bt[64:128, mc : mc + 1],
                            scalar2=None,
                            op0=ADD,
                        )
                    else:
                        nc.vector.tensor_copy(out=st[64:128, :], in_=pq[64:128, :])
                    nc.sync.dma_start(
                        out=dst[h_odd][0:64, ts(tq, 512)], in_=st[64:128, :]
                    )

                return (2000, emit)

            obs_tiles = {}

            def unit_outproj(qb, ec):
                def emit():
                    if ec == 0:
                        obs_tiles[qb] = obp.tile(
                            [P, 8, 512], BF16, tag="obs", name=f"obs{qb}"
                        )
                    obs = obs_tiles[qb]
                    po = pjp.tile([P, 512], F32, tag="pj", name=f"po{qb}_{ec}")
                    for k2 in range(2):
                        nc.tensor.matmul(
                            out=po[:],
                            lhsT=wo[:, k2, ds(ec * P, P)],
                            rhs=yt[k2][:, ts(qb, 512)],
                            start=(k2 == 0),
                            stop=(k2 == 1),
                        )
                    nc.vector.tensor_copy(out=obs[:, ec, :], in_=po[:])
                    if ec == 7:
                        nc.sync.dma_start(
                            out=out_d[:, :, ts(qb, 512)], in_=obs[:]
                        )

                return (700, emit)

            # attention backbone for one (qb, h): a list of units forming the
            # scores -> exp -> AV chain with AV one chunk behind, then the
            # denominator/normalize epilogue.
            def attn_units(qb, h):
                o = qb * 512
                jmax = qb * 4 + 4
                state = {}

                def mk_chunk(jc):
                    def emit():
                        if jc == 0:
                            state["py"] = psyp.tile(
                                [P, 512], F32, tag="psy", name=f"py{qb}_{h}"
                            )
                        py = state["py"]
                        r = jc * P - o
                        ps = pssp.tile(
                            [P, 512], F32, tag="pss", name=f"ps{qb}_{h}_{jc}"
                        )
                        if r < 0:
                            nc.tensor.matmul(
                                out=ps[:],
                                lhsT=ka[h][0:66, ts(jc, P)],
                                rhs=qa[h][0:66, ds(o, 512)],
                                start=True,
                                stop=True,
                            )
                            lo = 0
                        else:
                            lo = r
                            nc.tensor.matmul(
                                out=ps[:, lo:512],
                                lhsT=ka[h][0:66, ts(jc, P)],
                                rhs=qa[h][0:66, ds(o + lo, 512 - lo)],
                                start=True,
                                stop=False,
                            )
                            # causal stair: ps[:, r:r+128] += I.T @ maskst
                            nc.tensor.matmul(
                                out=ps[:, lo : lo + P],
                                lhsT=ident_sb[:],
                                rhs=maskst_sb[:],
                                start=False,
                                stop=True,
                            )
                        ex = expp.tile(
                            [P, 512], BF16, tag="ex", name=f"ex{qb}_{h}_{jc}"
                        )
                        nc.scalar.activation(
                            out=ex[:, lo:512], in_=ps[:, lo:512], func=EXP
                        )
                        if "pend" in state:
                            pjc, plo, pex = state["pend"]
                            nc.tensor.matmul(
                                out=py[:, plo:512],
                                lhsT=va[h][:, pjc, :],
                                rhs=pex[:, plo:512],
                                start=(pjc == 0),
                                stop=False,
                            )
                        state["pend"] = (jc, lo, ex)

                    cost = 512 if jc * P - o < 0 else (512 - (jc * P - o)) + 128
                    if jc > 0:
                        cost += 512  # trailing AV of the previous chunk
                    return (int(cost * 0.45), emit)

                def emit_tail():
                    py = state["py"]
                    pjc, plo, pex = state["pend"]
                    nc.tensor.matmul(
                        out=py[:, plo:512],
                        lhsT=va[h][:, pjc, :],
                        rhs=pex[:, plo:512],
                        start=(pjc == 0),
                        stop=True,
                    )

                def emit_norm():
                    py = state["py"]
                    dr = 64 if h % 2 == 0 else 32
                    rowbase = (h % 2) * 64
                    idx = qb * HPC + h
                    dn = small.tile([P, 512], F32, tag="dn", name=f"dn{qb}_{h}")
                    # approx recip mis-executes on partition-base slices;
                    # full-tile costs the same (DVE time ~ free size only).
                    # Junk rows of py produce junk reciprocals, never read.
                    nc.vector.reciprocal_approx_fast(out=dn[:], in_=py[:])
                    nc.sync.dma_start(
                        out=dscr_d[idx : idx + 1, :], in_=dn[dr : dr + 1, :]
                    )
                    rb = small.tile([P, 512], F32, tag="rb", name=f"rb{qb}_{h}")
                    nc.sync.dma_start(
                        out=rb[rowbase : rowbase + 64, :],
                        in_=dscr_d[idx : idx + 1, :].to_broadcast((64, 512)),
                    )
                    nc.vector.tensor_tensor(
                        out=yt[h // 2][rowbase : rowbase + 64, ds(o, 512)],
                        in0=py[rowbase : rowbase + 64, :],
                        in1=rb[rowbase : rowbase + 64, :],
                        op=MULT,
                    )

                units = [mk_chunk(jc) for jc in range(jmax)]
                units.append((230, emit_tail))
                units.append((100, emit_norm))
                return units

            def weave(backbone, fillers):
                tb = sum(c for c, _ in backbone) or 1
                tf = sum(c for c, _ in fillers) or 1
                ib = jf = 0
                cb = cf = 0.0
                while ib < len(backbone) or jf < len(fillers):
                    take_b = jf >= len(fillers) or (
                        ib < len(backbone) and cb / tb <= cf / tf
                    )
                    if take_b:
                        c, fn = backbone[ib]
                        ib += 1
                        cb += c
                    else:
                        c, fn = fillers[jf]
                        jf += 1
                        cf += c
                    fn()

            def proj_units(tq):
                us = []
                for ch in range(4):
                    us.append(unit_v(tq, ch))
                for mc in range(2):
                    us.append(unit_qk(tq, "q", mc))
                    us.append(unit_qk(tq, "k", mc))
                return us

            # ---- schedule: proj(0) | attn(0)+proj(1) | attn(1)+proj(2)+out(0)
            #      | attn(2)+proj(3)+out(1) | attn(3)+out(2) | out(3)
            for _, fn in proj_units(0):
                fn()
            for t in range(1, 4):
                backbone = []
                for h in range(HPC):
                    backbone += attn_units(t - 1, h)
                fillers = proj_units(t)
                if t >= 2:
                    fillers += [unit_outproj(t - 2, ec) for ec in range(8)]
                weave(backbone, fillers)
            backbone = []
            for h in range(HPC):
                backbone += attn_units(3, h)
            weave(backbone, [unit_outproj(2, ec) for ec in range(8)])
            for ec in range(8):
                unit_outproj(3, ec)[1]()

            if DEBUG:
                for h in range(HPC):
                    nc.sync.dma_start(out=qdump_d[h].bitcast(F32R), in_=qa[h][:])
                    nc.sync.dma_start(out=kdump_d[h].bitcast(F32R), in_=ka[h][:])
                    nc.sync.dma_start(
                        out=vdump_d[h], in_=va[h][:]
                    )
                for m in range(2):
                    nc.sync.dma_start(out=ydump_d[m], in_=yt[m][:])
                for tq in range(4):
                    nc.sync.dma_start(out=xdump_d[tq], in_=xb[tq][:])

    nc.compile()
    return nc


def _get_nc(with_bias: bool):
    key = (with_bias, DEBUG)
    if key not in _cache:
        _cache[key] = _build(with_bias)
    return _cache[key]


def kernel(x, freqs_cis, Wq, bq, Wkv, bkv, Wo, bo, **_unused):
    import ml_dtypes

    bf16 = ml_dtypes.bfloat16

    x = np.asarray(x, np.float32)
    Wq = np.asarray(Wq, np.float32)
    bq = np.asarray(bq, np.float32)
    Wkv = np.asarray(Wkv, np.float32)
    bkv = np.asarray(bkv, np.float32)
    Wo = np.asarray(Wo, np.float32)
    bo = np.asarray(bo, np.float32)

    with_bias = bool(np.any(bq) or np.any(bkv))
    nc = _get_nc(with_bias)

    scale = 1.0 / np.sqrt(DH)
    iota = np.arange(T, dtype=np.float32)

    # causal stair (applied via identity-matmul accumulation into PSUM):
    # maskst[p, m] = -1e30 where m < p (j = chunk base + p is in the future)
    mm = np.arange(P, dtype=np.float32)
    maskst = np.where(mm[None, :] < mm[:, None], NEG, 0.0).astype(bf16)
    ident = np.eye(P, dtype=bf16)

    kaug = np.stack([iota, np.ones(T, np.float32)])  # [2, T]

    # p-major packing: [...] -> [p, kc, t] so each x block / weight tensor
    # loads with ONE dma
    xT = [
        np.ascontiguousarray(x[b].T.reshape(8, P, T).transpose(1, 0, 2)).astype(bf16)
        for b in range(B)
    ]

    in_maps = []
    for c in range(N_CORES):
        b, g = divmod(c, G)
        rows = slice(g * DG, (g + 1) * DG)
        def pack(wT, n):  # [n*P, cols] -> [P, n, cols]
            return np.ascontiguousarray(
                wT.reshape(n, P, wT.shape[1]).transpose(1, 0, 2)
            ).astype(bf16)

        wqT = pack((Wq[rows] * scale).T, 8)
        wkT = pack(Wkv[0:D][rows].T, 8)
        wvT = pack(Wkv[D : 2 * D][rows].T, 8)
        woT = pack(Wo[:, rows].T, 2)
        qaug = np.zeros((HPC, 2, T), np.float32)
        for h in range(HPC):
            slope = (g * HPC + h + 1) / H
            qaug[h, 0, :] = slope
            qaug[h, 1, :] = -slope * iota
        m = {
            "xT": xT[b],
            "wqT": wqT,
            "wkT": wkT,
            "wvT": wvT,
            "woT": woT,
            "qaug": qaug,
            "kaug": kaug,
            "ident": ident,
            "maskst": maskst,
        }
        if with_bias:
            bv_g = bkv[D : 2 * D][rows]
            m["bvo"] = np.ascontiguousarray(
                np.broadcast_to(bv_g[None, :], (P, DG))
            ).astype(np.float32)
            m["bq2"] = np.ascontiguousarray((bq[rows] * scale).reshape(2, P).T)
            m["bk2"] = np.ascontiguousarray(bkv[0:D][rows].reshape(2, P).T)
        in_maps.append(m)

    res = run_bass_kernel_spmd(nc, in_maps, list(range(N_CORES)), trace=TRACE)
    global LAST_RESULTS
    LAST_RESULTS = res

    out = np.empty((B, T, D), np.float32)
    for b in range(B):
        acc = res.results[b * G]["outT"].astype(np.float32)  # [P, 8, T]
        for g in range(1, G):
            acc += res.results[b * G + g]["outT"].astype(np.float32)
        # [p, ec, t] -> [ec*128+p, t] -> [t, d]
        out[b] = acc.transpose(1, 0, 2).reshape(D, T).T + bo[None, :]
    return out
